# revision 1
# baseline (speedup 1.0000x reference)
"""Causal self-attention (B=2, T=2048, C=1024, 16 heads) on 8 TRN2 NeuronCores.

Sharding: 2-way data parallel (batch) x 4-way tensor parallel (heads).
Core c handles batch c//4 and heads [4*(c%4) .. 4*(c%4)+3].

Per-core pipeline (all matmuls bf16, fp32 PSUM accumulation):
  - host pre-transposes x[b] -> xT [C, T] bf16 so the contraction dim is
    on partitions everywhere (no on-device transposes needed).
  - qkT = [Wq|Wk]^T-style projection computed directly in transposed
    layout [j, T] (lhsT = weight columns, rhs = xT).  Q^T and K^T per
    head fall out as partition slices.
  - v computed in natural [T, d] layout (lhsT = xT chunks, rhs = Wv),
    stored per (t-tile, head) as [128, 65] with a ones-column appended
    so the PV matmul also produces the softmax denominator for free.
  - attention: S^T tiles [kblock=128, qblock=512] = K^T.T @ Q^T; exp on
    ScalarE (scale=1/8 folded in, no max subtraction -- scores are O(1)
    by construction); causal masking is multiplicative post-exp on the
    diagonal tiles only; off-causal tiles are skipped entirely.
    O^T [65, qblock] accumulates over kblocks in PSUM.
  - y^T = O^T[0:64] * recip(O^T[64]) broadcast (GpSimd partition
    broadcast), written as bf16 into the proj lhsT layout.
  - z_partial = y^T.T @ Wp_rows; AllReduce(add) over the 4 cores of the
    same batch, chunked by 512 query rows to overlap with compute.

Self-contained: hardcodes shapes; only imports the system concourse stack.
"""

import contextlib

import numpy as np
import ml_dtypes

B, T, C = 2, 2048, 1024
NH = 16
HS = 64
NCORES = 8
HPC = 4          # heads per core
CPC = HPC * HS   # channels per core (256)
P = 128
QB = 512         # query block (free dim of S^T / O^T tiles)
NQ = T // QB     # 4 query blocks
NTT = T // P     # 16 t-tiles / kblocks
KC = C // P      # 8 contraction chunks
GROUPS = [[0, 1, 2, 3], [4, 5, 6, 7]]

_CACHE = {}


def _build_nc():
    import concourse.tile as tile
    import concourse.mybir as mybir
    from concourse import bacc

    f32 = mybir.dt.float32
    bf16 = mybir.dt.bfloat16
    Alu = mybir.AluOpType
    Act = mybir.ActivationFunctionType

    nc = bacc.Bacc(
        "TRN2",
        target_bir_lowering=False,
        debug=False,
        enable_asserts=True,
        num_devices=NCORES,
    )
    xT = nc.dram_tensor("xT", [C, T], bf16, kind="ExternalInput").ap()
    wqk = nc.dram_tensor("wqk", [C, 2 * CPC], bf16, kind="ExternalInput").ap()
    wv = nc.dram_tensor("wv", [C, CPC], bf16, kind="ExternalInput").ap()
    wp = nc.dram_tensor("wp", [CPC, C], bf16, kind="ExternalInput").ap()
    bqk = nc.dram_tensor("bqk", [2 * CPC], f32, kind="ExternalInput").ap()
    bv = nc.dram_tensor("bv", [CPC], f32, kind="ExternalInput").ap()
    bp = nc.dram_tensor("bp", [C], f32, kind="ExternalInput").ap()
    out = nc.dram_tensor("out", [T, C], f32, kind="ExternalOutput").ap()

    with tile.TileContext(nc) as tc, contextlib.ExitStack() as ctx:
        consts = ctx.enter_context(tc.tile_pool(name="consts", bufs=1))
        big = ctx.enter_context(tc.tile_pool(name="big", bufs=1))
        ppool = ctx.enter_context(tc.tile_pool(name="ppool", bufs=6))
        zpool = ctx.enter_context(tc.tile_pool(name="zpool", bufs=4))
        rpool = ctx.enter_context(tc.tile_pool(name="rpool", bufs=4))
        ps_mm = ctx.enter_context(tc.tile_pool(name="ps_mm", bufs=2, space="PSUM"))
        ps_s = ctx.enter_context(tc.tile_pool(name="ps_s", bufs=3, space="PSUM"))
        ps_o = ctx.enter_context(tc.tile_pool(name="ps_o", bufs=2, space="PSUM"))
        dram = ctx.enter_context(tc.tile_pool(name="dram", bufs=2, space="DRAM"))

        # ---- constants ----
        wqk_sb = consts.tile([P, KC, 2 * CPC], bf16)
        nc.sync.dma_start(wqk_sb, wqk.rearrange("(o p) m -> p o m", p=P))
        wv_sb = consts.tile([P, KC, CPC], bf16)
        nc.sync.dma_start(wv_sb, wv.rearrange("(o p) m -> p o m", p=P))
        wp_sb = consts.tile([P, CPC // P, C], bf16)
        nc.sync.dma_start(wp_sb, wp.rearrange("(o p) m -> p o m", p=P))

        bqk_sb = consts.tile([P, 2 * CPC // P], f32)
        nc.sync.dma_start(bqk_sb, bqk.rearrange("(o p) -> p o", p=P))
        bv_row = consts.tile([1, CPC], f32)
        nc.sync.dma_start(bv_row, bv[None, :])
        bv_bc = consts.tile([P, CPC], f32)
        nc.gpsimd.partition_broadcast(bv_bc, bv_row)
        bp_row = consts.tile([1, C], f32)
        nc.sync.dma_start(bp_row, bp[None, :])
        bp_bc = consts.tile([P, C], f32)
        nc.gpsimd.partition_broadcast(bp_bc, bp_row)

        # multiplicative causal masks for the 4 diagonal-block offsets:
        # masks[r, p, c] = 1.0 if c >= 128*p + r else 0.0
        masks = consts.tile([P, 4, QB], bf16)
        nc.gpsimd.memset(masks, 1.0)
        for pos in range(4):
            nc.gpsimd.affine_select(
                out=masks[:, pos, :],
                in_=masks[:, pos, :],
                pattern=[[1, QB]],
                compare_op=Alu.is_ge,
                fill=0.0,
                base=-P * pos,
                channel_multiplier=-1,
            )

        # ---- persistent activations ----
        xT_sb = big.tile([P, KC, T], bf16)
        qk_sb = big.tile([P, 4, T], bf16)   # mi 0-1: q heads, 2-3: k heads
        y_sb = big.tile([P, CPC // P, T], bf16)
        v_sb = big.tile([P, NTT, HPC, 66], bf16)  # [.., 0:64]=v, [.., 64]=1.0
        nc.gpsimd.memset(v_sb, 1.0)

        xT_r = xT.rearrange("(o p) t -> p o t", p=P)

        for tc_i in range(NQ):
            tsl = slice(tc_i * QB, (tc_i + 1) * QB)
            # load this t-chunk of xT (all contraction chunks)
            for ci in range(KC):
                nc.sync.dma_start(xT_sb[:, ci, tsl], xT_r[:, ci, tsl])

            # qkT projection: [j, t] = wqk[:, j].T @ xT
            for mi in range(4):
                ps_qk = ps_mm.tile([P, QB], f32, tag="mm")
                for ci in range(KC):
                    nc.tensor.matmul(
                        ps_qk,
                        wqk_sb[:, ci, mi * P : (mi + 1) * P],
                        xT_sb[:, ci, tsl],
                        start=(ci == 0),
                        stop=(ci == KC - 1),
                    )
                nc.vector.tensor_scalar_add(
                    qk_sb[:, mi, tsl], ps_qk, bqk_sb[:, mi : mi + 1]
                )

            # v: [t, d] = xT[:, t].T @ wv, per t-tile, split per head
            for tl in range(4):
                tt = tc_i * 4 + tl
                ps_v = ps_mm.tile([P, CPC], f32, tag="mm")
                for ci in range(KC):
                    nc.tensor.matmul(
                        ps_v,
                        xT_sb[:, ci, tt * P : (tt + 1) * P],
                        wv_sb[:, ci, :],
                        start=(ci == 0),
                        stop=(ci == KC - 1),
                    )
                for h in range(HPC):
                    nc.vector.tensor_tensor(
                        v_sb[:, tt, h, 0:64],
                        ps_v[:, h * HS : (h + 1) * HS],
                        bv_bc[:, h * HS : (h + 1) * HS],
                        Alu.add,
                    )

            # attention for query block qi = tc_i (all needed K/V ready)
            qi = tc_i
            qsl = slice(qi * QB, (qi + 1) * QB)
            kmax = 4 * qi + 4
            for h in range(HPC):
                po = 64 * (h % 2)
                o_t = ps_o.tile([65, QB], f32, tag="o")
                for kb in range(kmax):
                    s_t = ps_s.tile([P, QB], f32, tag="s")
                    nc.tensor.matmul(
                        s_t,
                        qk_sb[po : po + 64, 2 + h // 2, kb * P : (kb + 1) * P],
                        qk_sb[po : po + 64, h // 2, qsl],
                        start=True,
                        stop=True,
                    )
                    p_t = ppool.tile([P, QB], bf16, tag="p")
                    nc.scalar.activation(p_t, s_t, Act.Exp, scale=0.125)
                    if kb >= 4 * qi:
                        nc.vector.tensor_tensor(
                            p_t, p_t, masks[:, kb - 4 * qi, :], Alu.mult
                        )
                    nc.tensor.matmul(
                        o_t,
                        v_sb[:, kb, h, 0:65],
                        p_t,
                        start=(kb == 0),
                        stop=(kb == kmax - 1),
                    )
                r_t = rpool.tile([1, QB], f32, tag="r")
                nc.vector.reciprocal(r_t, o_t[64:65, :])
                rb_t = rpool.tile([64, QB], f32, tag="rb")
                nc.gpsimd.partition_broadcast(rb_t, r_t)
                nc.vector.tensor_tensor(
                    y_sb[po : po + 64, h // 2, qsl], o_t[0:64, :], rb_t, Alu.mult
                )

            # projection for this chunk's 4 t-tiles, then chunked AllReduce
            z_loc = dram.tile([QB, C], f32, tag="zloc")
            for tl in range(4):
                tt = qi * 4 + tl
                for n in range(2):
                    ps_z = ps_mm.tile([P, QB], f32, tag="mm")
                    for kc2 in range(CPC // P):
                        nc.tensor.matmul(
                            ps_z,
                            y_sb[:, kc2, tt * P : (tt + 1) * P],
                            wp_sb[:, kc2, n * QB : (n + 1) * QB],
                            start=(kc2 == 0),
                            stop=(kc2 == CPC // P - 1),
                        )
                    z_t = zpool.tile([P, QB], f32, tag="z")
                    nc.vector.tensor_tensor(
                        z_t, ps_z, bp_bc[:, n * QB : (n + 1) * QB], Alu.add
                    )
                    nc.sync.dma_start(
                        z_loc[tl * P : (tl + 1) * P, n * QB : (n + 1) * QB], z_t
                    )
            z_red = dram.tile([QB, C], f32, tag="zred")
            nc.gpsimd.collective_compute(
                "AllReduce",
                Alu.add,
                replica_groups=GROUPS,
                ins=[z_loc.opt()],
                outs=[z_red.opt()],
            )
            nc.sync.dma_start(out[qsl, :], z_red)

    nc.compile()
    return nc


def get_nc():
    if "nc" not in _CACHE:
        _CACHE["nc"] = _build_nc()
    return _CACHE["nc"]


def make_in_maps(x, w_attn, b_attn, w_proj, b_proj):
    x = np.asarray(x, dtype=np.float32)
    w_attn = np.asarray(w_attn, dtype=np.float32)
    b_attn = np.asarray(b_attn, dtype=np.float32)
    w_proj = np.asarray(w_proj, dtype=np.float32)
    b_proj = np.asarray(b_proj, dtype=np.float32)
    bf = ml_dtypes.bfloat16

    in_maps = []
    for core in range(NCORES):
        b, g = core // 4, core % 4
        hsl = slice(g * CPC, (g + 1) * CPC)
        wq = w_attn[:, 0:C][:, hsl]
        wk = w_attn[:, C : 2 * C][:, hsl]
        wv_ = w_attn[:, 2 * C : 3 * C][:, hsl]
        in_maps.append(
            {
                "xT": np.ascontiguousarray(x[b].T).astype(bf),
                "wqk": np.ascontiguousarray(np.concatenate([wq, wk], axis=1)).astype(bf),
                "wv": np.ascontiguousarray(wv_).astype(bf),
                "wp": np.ascontiguousarray(w_proj[hsl, :]).astype(bf),
                "bqk": np.concatenate([b_attn[0:C][hsl], b_attn[C : 2 * C][hsl]]).astype(np.float32),
                "bv": np.ascontiguousarray(b_attn[2 * C : 3 * C][hsl]).astype(np.float32),
                "bp": b_proj.astype(np.float32),
            }
        )
    return in_maps


def kernel(x, w_attn, b_attn, w_proj, b_proj):
    from concourse.bass_utils import run_bass_kernel_spmd

    nc = get_nc()
    in_maps = make_in_maps(x, w_attn, b_attn, w_proj, b_proj)
    res = run_bass_kernel_spmd(nc, in_maps, core_ids=list(range(NCORES))).results
    out = np.empty((B, T, C), np.float32)
    out[0] = res[0]["out"]
    out[1] = res[4]["out"]
    return out
